# revision 1
# baseline (speedup 1.0000x reference)
"""Trainium2 Bass kernel for the GPT2Shared multimodal ensemble MLP.

Pipeline (per token): three modality adapters (Linear+GELU) -> shared
projection -> concat -> 32-expert ensemble MLP (2304->768->32->5, relu) ->
mean over experts -> mean over time.

Sharding: pure data-parallel over the batch dim. Each of the 8 cores gets
4 batches (1024 tokens) and runs the whole pipeline for its tokens; the
final reduction over experts+time happens on-device, so each core emits a
[5, 4] partial and the host only rescales/concats.

All on-device tensors live in [feature, token] layout so every matmul uses
the natural weight layout as the stationary operand and no transposes are
needed anywhere. Matmul operands are bf16 (PSUM accumulation is fp32);
host pre-casts/pre-transposes, which keeps HW DMA traffic halved.
"""

import os
import sys

for _p in ("/opt/trn_rl_repo", "/root/.axon_site/_ro/trn_rl_repo"):
    if os.path.isdir(_p) and _p not in sys.path:
        sys.path.append(_p)

import ml_dtypes
import numpy as np

import concourse.bass as bass
import concourse.tile as tile
from concourse import bacc, mybir
from concourse.bass_utils import run_bass_kernel_spmd

BF16 = mybir.dt.bfloat16
F32 = mybir.dt.float32
NPBF = ml_dtypes.bfloat16

N_CORES = 8
B, T = 32, 256
TOK = B * T // N_CORES          # 1024 tokens per core
BPC = B // N_CORES              # 4 batches per core
NT, NSZ = 2, 512                # token tiles per core
GKT = 18                        # 2304 gelu/chunk features = 18 k-tiles
PKT = 6                         # 768 proj features = 6 k-tiles
E, HID, TGT = 32, 32, 5
# (name, in_dim, in_ktiles, chunk row-tile offset) in reference concat order
# chunk = [video, text, audio]
MODS = (("v", 768, 6, 0), ("t", 768, 6, 6), ("a", 1024, 8, 12))

_NC = None
LAST_RESULT = None


def _build():
    nc = bacc.Bacc("TRN2", target_bir_lowering=False, debug=False,
                   num_devices=N_CORES)

    dr = {}
    for mn, kin, _, _ in MODS:
        dr[f"x{mn}"] = nc.dram_tensor(f"x{mn}", [kin, TOK], BF16, kind="ExternalInput")
        dr[f"W{mn}"] = nc.dram_tensor(f"W{mn}", [kin, 2304], BF16, kind="ExternalInput")
        dr[f"b{mn}"] = nc.dram_tensor(f"b{mn}", [128, GKT], F32, kind="ExternalInput")
    dr["Wp"] = nc.dram_tensor("Wp", [2304, 768], BF16, kind="ExternalInput")
    dr["bp"] = nc.dram_tensor("bp", [128, PKT], F32, kind="ExternalInput")
    dr["We1"] = nc.dram_tensor("We1", [E, 2304, 768], BF16, kind="ExternalInput")
    dr["be1"] = nc.dram_tensor("be1", [128, E, PKT], F32, kind="ExternalInput")
    # host-rearranged: [p, kt, e, h] <- We2[e, kt*128+p, h]
    dr["We2"] = nc.dram_tensor("We2", [128, PKT, E, HID], BF16, kind="ExternalInput")
    dr["be2"] = nc.dram_tensor("be2", [HID, E], F32, kind="ExternalInput")
    # host-stacked: [p, kt, t] <- We3[(kt*128+p)//32, (kt*128+p)%32, t]
    dr["We3"] = nc.dram_tensor("We3", [128, 8, TGT], BF16, kind="ExternalInput")
    out_d = nc.dram_tensor("out", [TGT, BPC], F32, kind="ExternalOutput")

    gelu = mybir.ActivationFunctionType.Gelu_apprx_tanh
    relu = mybir.ActivationFunctionType.Relu
    ident = mybir.ActivationFunctionType.Identity

    with tile.TileContext(nc) as tc:
        with (
            tc.tile_pool(name="const", bufs=1) as constp,
            tc.tile_pool(name="persist", bufs=1) as perp,
            tc.tile_pool(name="psA", bufs=4, space=bass.MemorySpace.PSUM) as psA,
        ):
            # small constants
            be1_sb = constp.tile([128, E, PKT], F32, tag="be1")
            nc.sync.dma_start(be1_sb[:], dr["be1"][:])
            we2_sb = constp.tile([128, PKT, E, HID], BF16, tag="we2")
            nc.sync.dma_start(we2_sb[:], dr["We2"][:])
            be2_sb = constp.tile([HID, E], F32, tag="be2")
            nc.sync.dma_start(be2_sb[:], dr["be2"][:])
            we3_sb = constp.tile([128, 8, TGT], BF16, tag="we3")
            nc.sync.dma_start(we3_sb[:], dr["We3"][:])
            bp_sb = constp.tile([128, PKT], F32, tag="bp")
            nc.sync.dma_start(bp_sb[:], dr["bp"][:])

            chunk_sb = perp.tile([128, GKT, TOK], BF16, tag="chunk")

            # ---------------- adapters + shared projection ----------------
            with (
                tc.tile_pool(name="adw", bufs=1) as adw,
                tc.tile_pool(name="adf", bufs=2) as adf,
            ):
                wp_sb = adw.tile([2304 // GKT * 0 + 128, GKT, 768], BF16, tag="wp")
                nc.sync.dma_start(
                    wp_sb[:], dr["Wp"].rearrange("(kt p) m -> p kt m", p=128))
                for mn, kin, kint, coff in MODS:
                    bm_sb = constp.tile([128, GKT], F32, tag=f"b{mn}")
                    nc.sync.dma_start(bm_sb[:], dr[f"b{mn}"][:])
                    wm_sb = adw.tile([128, 8, 2304], BF16, tag="wmod")
                    nc.sync.dma_start(
                        wm_sb[:, :kint, :],
                        dr[f"W{mn}"].rearrange("(kt p) m -> p kt m", p=128))
                    f_sb = adf.tile([128, 8, TOK], BF16, tag="feat")
                    nc.sync.dma_start(
                        f_sb[:, :kint, :],
                        dr[f"x{mn}"].rearrange("(kt p) n -> p kt n", p=128))
                    g_sb = adw.tile([128, GKT, TOK], BF16, tag="g")
                    # g = gelu(x @ Wm + bm), in [feature, token] layout
                    for n in range(NT):
                        for gf in range(GKT):
                            ps = psA.tile([128, NSZ], F32, tag="ps")
                            for kt in range(kint):
                                nc.tensor.matmul(
                                    ps[:],
                                    wm_sb[:, kt, gf * 128:(gf + 1) * 128],
                                    f_sb[:, kt, n * NSZ:(n + 1) * NSZ],
                                    start=(kt == 0), stop=(kt == kint - 1))
                            nc.scalar.activation(
                                g_sb[:, gf, n * NSZ:(n + 1) * NSZ], ps[:],
                                gelu, bias=bm_sb[:, gf:gf + 1])
                    # chunk rows [coff:coff+6] = g @ Wp + bp
                    for n in range(NT):
                        for pf in range(PKT):
                            ps = psA.tile([128, NSZ], F32, tag="ps")
                            for kt in range(GKT):
                                nc.tensor.matmul(
                                    ps[:],
                                    wp_sb[:, kt, pf * 128:(pf + 1) * 128],
                                    g_sb[:, kt, n * NSZ:(n + 1) * NSZ],
                                    start=(kt == 0), stop=(kt == GKT - 1))
                            nc.scalar.activation(
                                chunk_sb[:, coff + pf, n * NSZ:(n + 1) * NSZ],
                                ps[:], ident, bias=bp_sb[:, pf:pf + 1])

            # ---------------- ensemble ----------------
            h2_sb = perp.tile([128, 8, TOK], BF16, tag="h2")
            with (
                tc.tile_pool(name="we1p", bufs=2) as we1p,
                tc.tile_pool(name="h1p", bufs=2) as h1p,
                tc.tile_pool(name="psB", bufs=2, space=bass.MemorySpace.PSUM) as psB,
            ):
                for e in range(E):
                    w1_sb = we1p.tile([128, GKT, 768], BF16, tag="w1")
                    nc.sync.dma_start(
                        w1_sb[:],
                        dr["We1"][e].rearrange("(kt p) m -> p kt m", p=128))
                    h1_sb = h1p.tile([128, PKT, TOK], BF16, tag="h1")
                    for n in range(NT):
                        for pf in range(PKT):
                            ps = psA.tile([128, NSZ], F32, tag="ps")
                            for kt in range(GKT):
                                nc.tensor.matmul(
                                    ps[:],
                                    w1_sb[:, kt, pf * 128:(pf + 1) * 128],
                                    chunk_sb[:, kt, n * NSZ:(n + 1) * NSZ],
                                    start=(kt == 0), stop=(kt == GKT - 1))
                            nc.scalar.activation(
                                h1_sb[:, pf, n * NSZ:(n + 1) * NSZ], ps[:],
                                relu, bias=be1_sb[:, e, pf:pf + 1])
                    for n in range(NT):
                        ps2 = psB.tile([HID, NSZ], F32, tag="ps2")
                        for kt in range(PKT):
                            nc.tensor.matmul(
                                ps2[:],
                                we2_sb[:, kt, e, :],
                                h1_sb[:, kt, n * NSZ:(n + 1) * NSZ],
                                start=(kt == 0), stop=(kt == PKT - 1))
                        q = e % 4
                        nc.scalar.activation(
                            h2_sb[q * 32:(q + 1) * 32, e // 4,
                                  n * NSZ:(n + 1) * NSZ],
                            ps2[:], relu, bias=be2_sb[:, e:e + 1])

                # ensemble head: accumulate all 32 experts' 5-dim outputs and
                # reduce over time within each batch
                s_sb = constp.tile([TGT, BPC], F32, tag="s")
                for n in range(NT):
                    ps3 = psB.tile([TGT, NSZ], F32, tag="ps3")
                    for kt in range(8):
                        nc.tensor.matmul(
                            ps3[:],
                            we3_sb[:, kt, :],
                            h2_sb[:, kt, n * NSZ:(n + 1) * NSZ],
                            start=(kt == 0), stop=(kt == 7))
                    nc.vector.reduce_sum(
                        s_sb[:, 2 * n:2 * n + 2],
                        ps3[:].rearrange("p (g t) -> p g t", t=T),
                        axis=mybir.AxisListType.X)
                nc.sync.dma_start(out_d[:], s_sb[:])

    nc.compile()
    return nc


def _prep(inputs):
    """Host-side: cast to bf16, transpose feats to [feature, token], build
    per-core input maps."""
    f32 = np.float32

    def bf(x):
        return np.asarray(x, f32).astype(NPBF)

    feats = {
        "v": np.asarray(inputs["video_feat"], f32).reshape(B * T, 768),
        "t": np.asarray(inputs["text_feat"], f32).reshape(B * T, 768),
        "a": np.asarray(inputs["audio_feat"], f32).reshape(B * T, 1024),
    }
    featsT = {k: bf(v.T) for k, v in feats.items()}

    wkeys = {"v": "Wv", "t": "Wt", "a": "Wa"}
    bkeys = {"v": "bv", "t": "bt", "a": "ba"}
    shared = {}
    for mn, kin, _, _ in MODS:
        shared[f"W{mn}"] = bf(inputs[wkeys[mn]])
        shared[f"b{mn}"] = np.ascontiguousarray(
            np.asarray(inputs[bkeys[mn]], f32).reshape(GKT, 128).T)
    shared["Wp"] = bf(inputs["Wp"])
    shared["bp"] = np.ascontiguousarray(
        np.asarray(inputs["bp"], f32).reshape(PKT, 128).T)
    shared["We1"] = bf(inputs["We1"])
    shared["be1"] = np.ascontiguousarray(
        np.asarray(inputs["be1"], f32).reshape(E, PKT, 128).transpose(2, 0, 1))
    shared["We2"] = np.ascontiguousarray(
        bf(inputs["We2"]).reshape(E, PKT, 128, HID).transpose(2, 1, 0, 3))
    shared["be2"] = np.ascontiguousarray(np.asarray(inputs["be2"], f32).T)
    shared["We3"] = np.ascontiguousarray(
        bf(inputs["We3"]).reshape(8, 128, TGT).transpose(1, 0, 2))

    in_maps = []
    for c in range(N_CORES):
        m = dict(shared)
        sl = slice(c * TOK, (c + 1) * TOK)
        for mn, _, _, _ in MODS:
            m[f"x{mn}"] = np.ascontiguousarray(featsT[mn][:, sl])
        in_maps.append(m)
    be3_sum = np.asarray(inputs["be3"], f32).sum(axis=0)
    return in_maps, be3_sum


def kernel(**inputs):
    global _NC, LAST_RESULT
    if _NC is None:
        _NC = _build()
    in_maps, be3_sum = _prep(inputs)
    trace = bool(os.environ.get("BASS_KERNEL_TRACE"))
    kwargs = {}
    if trace:
        import concourse.bass_utils as _bu
        _bu.upload_artifacts = lambda d: d  # no artifact bucket here
        kwargs["tmpdir"] = os.environ.get("BASS_KERNEL_TRACE_DIR") or None
    res = run_bass_kernel_spmd(_NC, in_maps, list(range(N_CORES)),
                               trace=trace, **kwargs)
    LAST_RESULT = res
    logits = np.empty((B, TGT), np.float32)
    for c in range(N_CORES):
        s = res.results[c]["out"]  # [TGT, BPC]
        logits[c * BPC:(c + 1) * BPC] = ((s + be3_sum[:, None] * T) / (E * T)).T
    return logits



# revision 2
# speedup vs baseline: 1.8732x; 1.8732x over previous
"""Trainium2 Bass kernel for the GPT2Shared multimodal ensemble MLP.

Pipeline (per token): three modality adapters (Linear+GELU) -> shared
projection -> concat -> 32-expert ensemble MLP (2304->768->32->5, relu) ->
mean over experts -> mean over time.

Sharding: pure data-parallel over the batch dim. Each of the 8 cores gets
4 batches (1024 tokens) and runs the whole pipeline for its tokens; the
final reduction over experts+time happens on-device, so each core emits a
[5, 4] partial and the host only rescales/concats.

All on-device tensors live in [feature, token] layout so every matmul uses
the natural weight layout as the stationary operand and no transposes are
needed anywhere.

Precision: the adapter, projection and ensemble-L1 matmuls (97% of the
MACs) run in fp8 e4m3 with MatmulPerfMode.DoubleRow (2x PE throughput,
256-deep contraction per instruction). Weights are pre-scaled x32 on the
host so they sit in e4m3's normal range; the dequant is folded into each
activation's scale operand. The chunk activations are stored x4 in fp8
(folded the same way; relu/identity are homogeneous). The small ensemble
L2/L3 layers stay bf16 - they are <10% of the compute but dominate the
fp8 quantization error of the final logits.
"""

import os
import sys

for _p in ("/opt/trn_rl_repo", "/root/.axon_site/_ro/trn_rl_repo"):
    if os.path.isdir(_p) and _p not in sys.path:
        sys.path.append(_p)

import ml_dtypes
import numpy as np

import concourse.bass as bass
import concourse.tile as tile
from concourse import bacc, mybir
from concourse.bass_utils import run_bass_kernel_spmd

BF16 = mybir.dt.bfloat16
F8 = mybir.dt.float8e4
F32 = mybir.dt.float32
NPBF = ml_dtypes.bfloat16
NPF8 = ml_dtypes.float8_e4m3

N_CORES = 8
B, T = 32, 256
TOK = B * T // N_CORES          # 1024 tokens per core
BPC = B // N_CORES              # 4 batches per core
NT, NSZ = 2, 512                # token tiles per core
GKT = 18                        # 2304 gelu/chunk features = 18 k-tiles
PKT = 6                         # 768 proj features = 6 k-tiles
E, HID, TGT = 32, 32, 5
SW = 32.0                       # host-side fp8 weight pre-scale
SC = 4.0                        # fp8 storage scale of the chunk activations
# (name, in_dim, in_ktiles, chunk row-tile offset) in reference concat order
# chunk = [video, text, audio]
MODS = (("v", 768, 6, 0), ("t", 768, 6, 6), ("a", 1024, 8, 12))

_NC = None
LAST_RESULT = None


def _build():
    nc = bacc.Bacc("TRN2", target_bir_lowering=False, debug=False,
                   num_devices=N_CORES)

    dr = {}
    for mn, kin, _, _ in MODS:
        dr[f"x{mn}"] = nc.dram_tensor(f"x{mn}", [kin, TOK], F8, kind="ExternalInput")
        dr[f"W{mn}"] = nc.dram_tensor(f"W{mn}", [kin, 2304], F8, kind="ExternalInput")
        dr[f"b{mn}"] = nc.dram_tensor(f"b{mn}", [128, GKT], F32, kind="ExternalInput")
    dr["Wp"] = nc.dram_tensor("Wp", [2304, 768], F8, kind="ExternalInput")
    dr["bp"] = nc.dram_tensor("bp", [128, PKT], F32, kind="ExternalInput")  # 4*bp
    dr["We1"] = nc.dram_tensor("We1", [E, 2304, 768], F8, kind="ExternalInput")
    dr["be1"] = nc.dram_tensor("be1", [128, E, PKT], F32, kind="ExternalInput")
    # host-rearranged: [p, kt, e, h] <- We2[e, kt*128+p, h]
    dr["We2"] = nc.dram_tensor("We2", [128, PKT, E, HID], BF16, kind="ExternalInput")
    dr["be2"] = nc.dram_tensor("be2", [HID, E], F32, kind="ExternalInput")
    # host-stacked: [p, kt, t] <- We3[(kt*128+p)//32, (kt*128+p)%32, t]
    dr["We3"] = nc.dram_tensor("We3", [128, 8, TGT], BF16, kind="ExternalInput")
    out_d = nc.dram_tensor("out", [TGT, BPC], F32, kind="ExternalOutput")

    gelu = mybir.ActivationFunctionType.Gelu_apprx_tanh
    relu = mybir.ActivationFunctionType.Relu
    ident = mybir.ActivationFunctionType.Identity
    DR = mybir.MatmulPerfMode.DoubleRow

    with tile.TileContext(nc) as tc:
        with (
            tc.tile_pool(name="const", bufs=1) as constp,
            tc.tile_pool(name="persist", bufs=1) as perp,
            tc.tile_pool(name="psA", bufs=4, space=bass.MemorySpace.PSUM) as psA,
        ):
            # small constants
            be1_sb = constp.tile([128, E, PKT], F32, tag="be1")
            nc.sync.dma_start(be1_sb[:], dr["be1"][:])
            we2_sb = constp.tile([128, PKT, E, HID], BF16, tag="we2")
            nc.sync.dma_start(we2_sb[:], dr["We2"][:])
            be2_sb = constp.tile([HID, E], F32, tag="be2")
            nc.sync.dma_start(be2_sb[:], dr["be2"][:])
            we3_sb = constp.tile([128, 8, TGT], BF16, tag="we3")
            nc.sync.dma_start(we3_sb[:], dr["We3"][:])
            bp_sb = constp.tile([128, PKT], F32, tag="bp")
            nc.sync.dma_start(bp_sb[:], dr["bp"][:])

            chunk_sb = perp.tile([128, GKT, TOK], F8, tag="chunk")

            # ---------------- adapters + shared projection ----------------
            with (
                tc.tile_pool(name="adw", bufs=1) as adw,
                tc.tile_pool(name="adf", bufs=2) as adf,
            ):
                wp_sb = adw.tile([128, GKT, 768], F8, tag="wp")
                nc.sync.dma_start(
                    wp_sb[:], dr["Wp"].rearrange("(kt p) m -> p kt m", p=128))
                for mn, kin, kint, coff in MODS:
                    bm_sb = constp.tile([128, GKT], F32, tag=f"b{mn}")
                    nc.sync.dma_start(bm_sb[:], dr[f"b{mn}"][:])
                    wm_sb = adw.tile([128, 8, 2304], F8, tag="wmod")
                    nc.sync.dma_start(
                        wm_sb[:, :kint, :],
                        dr[f"W{mn}"].rearrange("(kt p) m -> p kt m", p=128))
                    f_sb = adf.tile([128, 8, TOK], F8, tag="feat")
                    nc.sync.dma_start(
                        f_sb[:, :kint, :],
                        dr[f"x{mn}"].rearrange("(kt p) n -> p kt n", p=128))
                    g_sb = adw.tile([128, GKT, TOK], F8, tag="g")
                    # g = gelu(x @ Wm + bm), in [feature, token] layout
                    for n in range(NT):
                        for gf in range(GKT):
                            ps = psA.tile([128, NSZ], F32, tag="ps")
                            for j in range(kint // 2):
                                nc.tensor.matmul(
                                    ps[:],
                                    wm_sb[:, 2 * j:2 * j + 2,
                                          gf * 128:(gf + 1) * 128],
                                    f_sb[:, 2 * j:2 * j + 2,
                                         n * NSZ:(n + 1) * NSZ],
                                    start=(j == 0), stop=(j == kint // 2 - 1),
                                    perf_mode=DR)
                            nc.scalar.activation(
                                g_sb[:, gf, n * NSZ:(n + 1) * NSZ], ps[:],
                                gelu, bias=bm_sb[:, gf:gf + 1], scale=1.0 / SW)
                    # chunk rows [coff:coff+6] = SC * (g @ Wp + bp)
                    for n in range(NT):
                        for pf in range(PKT):
                            ps = psA.tile([128, NSZ], F32, tag="ps")
                            for j in range(GKT // 2):
                                nc.tensor.matmul(
                                    ps[:],
                                    wp_sb[:, 2 * j:2 * j + 2,
                                          pf * 128:(pf + 1) * 128],
                                    g_sb[:, 2 * j:2 * j + 2,
                                         n * NSZ:(n + 1) * NSZ],
                                    start=(j == 0), stop=(j == GKT // 2 - 1),
                                    perf_mode=DR)
                            nc.scalar.activation(
                                chunk_sb[:, coff + pf, n * NSZ:(n + 1) * NSZ],
                                ps[:], ident, bias=bp_sb[:, pf:pf + 1],
                                scale=SC / SW)

            # ---------------- ensemble ----------------
            h2_sb = perp.tile([128, 8, TOK], BF16, tag="h2")
            with (
                tc.tile_pool(name="we1p", bufs=2) as we1p,
                tc.tile_pool(name="h1p", bufs=2) as h1p,
                tc.tile_pool(name="psB", bufs=2, space=bass.MemorySpace.PSUM) as psB,
            ):
                for e in range(E):
                    w1_sb = we1p.tile([128, GKT, 768], F8, tag="w1")
                    nc.sync.dma_start(
                        w1_sb[:],
                        dr["We1"][e].rearrange("(kt p) m -> p kt m", p=128))
                    h1_sb = h1p.tile([128, PKT, TOK], BF16, tag="h1")
                    for n in range(NT):
                        for pf in range(PKT):
                            ps = psA.tile([128, NSZ], F32, tag="ps")
                            for j in range(GKT // 2):
                                nc.tensor.matmul(
                                    ps[:],
                                    w1_sb[:, 2 * j:2 * j + 2,
                                          pf * 128:(pf + 1) * 128],
                                    chunk_sb[:, 2 * j:2 * j + 2,
                                             n * NSZ:(n + 1) * NSZ],
                                    start=(j == 0), stop=(j == GKT // 2 - 1),
                                    perf_mode=DR)
                            nc.scalar.activation(
                                h1_sb[:, pf, n * NSZ:(n + 1) * NSZ], ps[:],
                                relu, bias=be1_sb[:, e, pf:pf + 1],
                                scale=1.0 / (SW * SC))
                    for n in range(NT):
                        ps2 = psB.tile([HID, NSZ], F32, tag="ps2")
                        for kt in range(PKT):
                            nc.tensor.matmul(
                                ps2[:],
                                we2_sb[:, kt, e, :],
                                h1_sb[:, kt, n * NSZ:(n + 1) * NSZ],
                                start=(kt == 0), stop=(kt == PKT - 1))
                        q = e % 4
                        nc.scalar.activation(
                            h2_sb[q * 32:(q + 1) * 32, e // 4,
                                  n * NSZ:(n + 1) * NSZ],
                            ps2[:], relu, bias=be2_sb[:, e:e + 1])

                # ensemble head: accumulate all 32 experts' 5-dim outputs and
                # reduce over time within each batch
                s_sb = constp.tile([TGT, BPC], F32, tag="s")
                for n in range(NT):
                    ps3 = psB.tile([TGT, NSZ], F32, tag="ps3")
                    for kt in range(8):
                        nc.tensor.matmul(
                            ps3[:],
                            we3_sb[:, kt, :],
                            h2_sb[:, kt, n * NSZ:(n + 1) * NSZ],
                            start=(kt == 0), stop=(kt == 7))
                    nc.vector.reduce_sum(
                        s_sb[:, 2 * n:2 * n + 2],
                        ps3[:].rearrange("p (g t) -> p g t", t=T),
                        axis=mybir.AxisListType.X)
                nc.sync.dma_start(out_d[:], s_sb[:])

    nc.compile()
    return nc


def _prep(inputs):
    """Host-side: quantize/cast, transpose feats to [feature, token], build
    per-core input maps."""
    f32 = np.float32

    def bf(x):
        return np.asarray(x, f32).astype(NPBF)

    def q8(x, s=1.0):
        return (np.asarray(x, f32) * f32(s)).astype(NPF8)

    feats = {
        "v": np.asarray(inputs["video_feat"], f32).reshape(B * T, 768),
        "t": np.asarray(inputs["text_feat"], f32).reshape(B * T, 768),
        "a": np.asarray(inputs["audio_feat"], f32).reshape(B * T, 1024),
    }
    featsT = {k: q8(v.T) for k, v in feats.items()}

    wkeys = {"v": "Wv", "t": "Wt", "a": "Wa"}
    bkeys = {"v": "bv", "t": "bt", "a": "ba"}
    shared = {}
    for mn, kin, _, _ in MODS:
        shared[f"W{mn}"] = q8(inputs[wkeys[mn]], SW)
        shared[f"b{mn}"] = np.ascontiguousarray(
            np.asarray(inputs[bkeys[mn]], f32).reshape(GKT, 128).T)
    shared["Wp"] = q8(inputs["Wp"], SW)
    shared["bp"] = np.ascontiguousarray(
        np.asarray(inputs["bp"], f32).reshape(PKT, 128).T * f32(SC))
    shared["We1"] = q8(inputs["We1"], SW)
    shared["be1"] = np.ascontiguousarray(
        np.asarray(inputs["be1"], f32).reshape(E, PKT, 128).transpose(2, 0, 1))
    shared["We2"] = np.ascontiguousarray(
        bf(inputs["We2"]).reshape(E, PKT, 128, HID).transpose(2, 1, 0, 3))
    shared["be2"] = np.ascontiguousarray(np.asarray(inputs["be2"], f32).T)
    shared["We3"] = np.ascontiguousarray(
        bf(inputs["We3"]).reshape(8, 128, TGT).transpose(1, 0, 2))

    in_maps = []
    for c in range(N_CORES):
        m = dict(shared)
        sl = slice(c * TOK, (c + 1) * TOK)
        for mn, _, _, _ in MODS:
            m[f"x{mn}"] = np.ascontiguousarray(featsT[mn][:, sl])
        in_maps.append(m)
    be3_sum = np.asarray(inputs["be3"], f32).sum(axis=0)
    return in_maps, be3_sum


def kernel(**inputs):
    global _NC, LAST_RESULT
    if _NC is None:
        _NC = _build()
    in_maps, be3_sum = _prep(inputs)
    trace = bool(os.environ.get("BASS_KERNEL_TRACE"))
    kwargs = {}
    if trace:
        import concourse.bass_utils as _bu
        _bu.upload_artifacts = lambda d: d  # no artifact bucket here
        kwargs["tmpdir"] = os.environ.get("BASS_KERNEL_TRACE_DIR") or None
    res = run_bass_kernel_spmd(_NC, in_maps, list(range(N_CORES)),
                               trace=trace, **kwargs)
    LAST_RESULT = res
    logits = np.empty((B, TGT), np.float32)
    for c in range(N_CORES):
        s = res.results[c]["out"]  # [TGT, BPC]
        logits[c * BPC:(c + 1) * BPC] = ((s + be3_sum[:, None] * T) / (E * T)).T
    return logits


# revision 5
# speedup vs baseline: 1.8769x; 1.0020x over previous
"""Trainium2 Bass kernel for the GPT2Shared multimodal ensemble MLP.

Pipeline (per token): three modality adapters (Linear+GELU) -> shared
projection -> concat -> 32-expert ensemble MLP (2304->768->32->5, relu) ->
mean over experts -> mean over time.

Sharding: pure data-parallel over the batch dim. Each of the 8 cores gets
4 batches (1024 tokens) and runs the whole pipeline for its tokens; the
final reduction over experts+time happens on-device, so each core emits a
[5, 4] partial and the host only rescales/concats.

All on-device tensors live in [feature, token] layout so every matmul uses
the natural weight layout as the stationary operand and no transposes are
needed anywhere.

Precision: the adapter, projection and ensemble-L1 matmuls (97% of the
MACs) run in fp8 e4m3 with MatmulPerfMode.DoubleRow (2x PE throughput,
256-deep contraction per instruction). Weights are pre-scaled x32 on the
host so they sit in e4m3's normal range; the dequant is folded into each
activation's scale operand. The chunk activations are stored x4 in fp8
(folded the same way; relu/identity are homogeneous). The small ensemble
L2/L3 layers stay bf16 - they are <10% of the compute but dominate the
fp8 quantization error of the final logits.
"""

import os
import sys

for _p in ("/opt/trn_rl_repo", "/root/.axon_site/_ro/trn_rl_repo"):
    if os.path.isdir(_p) and _p not in sys.path:
        sys.path.append(_p)

import ml_dtypes
import numpy as np

import concourse.bass as bass
import concourse.tile as tile
from concourse import bacc, mybir
from concourse.bass_utils import run_bass_kernel_spmd

BF16 = mybir.dt.bfloat16
F8 = mybir.dt.float8e4
F32 = mybir.dt.float32
NPBF = ml_dtypes.bfloat16
NPF8 = ml_dtypes.float8_e4m3

N_CORES = 8
B, T = 32, 256
TOK = B * T // N_CORES          # 1024 tokens per core
BPC = B // N_CORES              # 4 batches per core
NT, NSZ = 2, 512                # token tiles per core
GKT = 18                        # 2304 gelu/chunk features = 18 k-tiles
PKT = 6                         # 768 proj features = 6 k-tiles
E, HID, TGT = 32, 32, 5
SW = 32.0                       # host-side fp8 weight pre-scale
SC = 4.0                        # fp8 storage scale of the chunk activations
# (name, in_dim, in_ktiles, chunk row-tile offset) in reference concat order
# chunk = [video, text, audio]
MODS = (("v", 768, 6, 0), ("t", 768, 6, 6), ("a", 1024, 8, 12))

_NC = None
LAST_RESULT = None


def _build():
    nc = bacc.Bacc("TRN2", target_bir_lowering=False, debug=False,
                   num_devices=N_CORES)

    dr = {}
    for mn, kin, _, _ in MODS:
        dr[f"x{mn}"] = nc.dram_tensor(f"x{mn}", [kin, TOK], F8, kind="ExternalInput")
        dr[f"W{mn}"] = nc.dram_tensor(f"W{mn}", [kin, 2304], F8, kind="ExternalInput")
        dr[f"b{mn}"] = nc.dram_tensor(f"b{mn}", [128, GKT], F32, kind="ExternalInput")
    dr["Wp"] = nc.dram_tensor("Wp", [2304, 768], F8, kind="ExternalInput")
    dr["bp"] = nc.dram_tensor("bp", [128, PKT], F32, kind="ExternalInput")  # 4*bp
    dr["We1"] = nc.dram_tensor("We1", [E, 2304, 768], F8, kind="ExternalInput")
    dr["be1"] = nc.dram_tensor("be1", [128, E, PKT], F32, kind="ExternalInput")
    # host-rearranged: [p, kt, e, h] <- We2[e, kt*128+p, h]
    dr["We2"] = nc.dram_tensor("We2", [128, PKT, E, HID], BF16, kind="ExternalInput")
    dr["be2"] = nc.dram_tensor("be2", [HID, E], F32, kind="ExternalInput")
    # host-stacked: [p, kt, t] <- We3[(kt*128+p)//32, (kt*128+p)%32, t]
    dr["We3"] = nc.dram_tensor("We3", [128, 8, TGT], BF16, kind="ExternalInput")
    out_d = nc.dram_tensor("out", [TGT, BPC], F32, kind="ExternalOutput")

    gelu = mybir.ActivationFunctionType.Gelu_apprx_tanh
    relu = mybir.ActivationFunctionType.Relu
    ident = mybir.ActivationFunctionType.Identity
    DR = mybir.MatmulPerfMode.DoubleRow

    with tile.TileContext(nc) as tc:
        with (
            tc.tile_pool(name="const", bufs=1) as constp,
            tc.tile_pool(name="persist", bufs=1) as perp,
            tc.tile_pool(name="psA", bufs=4, space=bass.MemorySpace.PSUM) as psA,
        ):
            # small constants
            be1_sb = constp.tile([128, E, PKT], F32, tag="be1")
            nc.sync.dma_start(be1_sb[:], dr["be1"][:])
            we2_sb = constp.tile([128, PKT, E, HID], BF16, tag="we2")
            nc.sync.dma_start(we2_sb[:], dr["We2"][:])
            be2_sb = constp.tile([HID, E], F32, tag="be2")
            nc.sync.dma_start(be2_sb[:], dr["be2"][:])
            we3_sb = constp.tile([128, 8, TGT], BF16, tag="we3")
            nc.sync.dma_start(we3_sb[:], dr["We3"][:])
            bp_sb = constp.tile([128, PKT], F32, tag="bp")
            nc.sync.dma_start(bp_sb[:], dr["bp"][:])

            chunk_sb = perp.tile([128, GKT, TOK], F8, tag="chunk")

            # ---------------- adapters + shared projection ----------------
            with (
                tc.tile_pool(name="adw", bufs=1) as adw,
                tc.tile_pool(name="adf", bufs=2) as adf,
            ):
                wp_sb = adw.tile([128, GKT, 768], F8, tag="wp")
                nc.sync.dma_start(
                    wp_sb[:], dr["Wp"].rearrange("(kt p) m -> p kt m", p=128))
                for mn, kin, kint, coff in MODS:
                    bm_sb = constp.tile([128, GKT], F32, tag=f"b{mn}")
                    nc.sync.dma_start(bm_sb[:], dr[f"b{mn}"][:])
                    wm_sb = adw.tile([128, 8, 2304], F8, tag="wmod")
                    nc.sync.dma_start(
                        wm_sb[:, :kint, :],
                        dr[f"W{mn}"].rearrange("(kt p) m -> p kt m", p=128))
                    f_sb = adf.tile([128, 8, TOK], F8, tag="feat")
                    nc.sync.dma_start(
                        f_sb[:, :kint, :],
                        dr[f"x{mn}"].rearrange("(kt p) n -> p kt n", p=128))
                    g_sb = adw.tile([128, GKT, TOK], F8, tag="g")
                    # g = gelu(x @ Wm + bm), in [feature, token] layout
                    for n in range(NT):
                        for gf in range(GKT):
                            ps = psA.tile([128, NSZ], F32, tag="ps")
                            for j in range(kint // 2):
                                nc.tensor.matmul(
                                    ps[:],
                                    wm_sb[:, 2 * j:2 * j + 2,
                                          gf * 128:(gf + 1) * 128],
                                    f_sb[:, 2 * j:2 * j + 2,
                                         n * NSZ:(n + 1) * NSZ],
                                    start=(j == 0), stop=(j == kint // 2 - 1),
                                    perf_mode=DR)
                            nc.scalar.activation(
                                g_sb[:, gf, n * NSZ:(n + 1) * NSZ], ps[:],
                                gelu, bias=bm_sb[:, gf:gf + 1], scale=1.0 / SW)
                    # chunk rows [coff:coff+6] = SC * (g @ Wp + bp)
                    for n in range(NT):
                        for pf in range(PKT):
                            ps = psA.tile([128, NSZ], F32, tag="ps")
                            for j in range(GKT // 2):
                                nc.tensor.matmul(
                                    ps[:],
                                    wp_sb[:, 2 * j:2 * j + 2,
                                          pf * 128:(pf + 1) * 128],
                                    g_sb[:, 2 * j:2 * j + 2,
                                         n * NSZ:(n + 1) * NSZ],
                                    start=(j == 0), stop=(j == GKT // 2 - 1),
                                    perf_mode=DR)
                            nc.scalar.activation(
                                chunk_sb[:, coff + pf, n * NSZ:(n + 1) * NSZ],
                                ps[:], ident, bias=bp_sb[:, pf:pf + 1],
                                scale=SC / SW)

            # ---------------- ensemble ----------------
            h2_sb = perp.tile([128, 8, TOK], BF16, tag="h2")
            with (
                tc.tile_pool(name="we1p", bufs=3) as we1p,
                tc.tile_pool(name="h1p", bufs=3) as h1p,
                tc.tile_pool(name="psB", bufs=2, space=bass.MemorySpace.PSUM) as psB,
            ):
                for e in range(E):
                    w1_sb = we1p.tile([128, GKT, 768], F8, tag="w1")
                    nc.sync.dma_start(
                        w1_sb[:],
                        dr["We1"][e].rearrange("(kt p) m -> p kt m", p=128))
                    h1_sb = h1p.tile([128, PKT, TOK], BF16, tag="h1")
                    for n in range(NT):
                        for pf in range(PKT):
                            ps = psA.tile([128, NSZ], F32, tag="ps")
                            for j in range(GKT // 2):
                                nc.tensor.matmul(
                                    ps[:],
                                    w1_sb[:, 2 * j:2 * j + 2,
                                          pf * 128:(pf + 1) * 128],
                                    chunk_sb[:, 2 * j:2 * j + 2,
                                             n * NSZ:(n + 1) * NSZ],
                                    start=(j == 0), stop=(j == GKT // 2 - 1),
                                    perf_mode=DR)
                            nc.scalar.activation(
                                h1_sb[:, pf, n * NSZ:(n + 1) * NSZ], ps[:],
                                relu, bias=be1_sb[:, e, pf:pf + 1],
                                scale=1.0 / (SW * SC))
                    for n in range(NT):
                        ps2 = psB.tile([HID, NSZ], F32, tag="ps2")
                        for kt in range(PKT):
                            nc.tensor.matmul(
                                ps2[:],
                                we2_sb[:, kt, e, :],
                                h1_sb[:, kt, n * NSZ:(n + 1) * NSZ],
                                start=(kt == 0), stop=(kt == PKT - 1))
                        q = e % 4
                        nc.scalar.activation(
                            h2_sb[q * 32:(q + 1) * 32, e // 4,
                                  n * NSZ:(n + 1) * NSZ],
                            ps2[:], relu, bias=be2_sb[:, e:e + 1])

                # ensemble head: accumulate all 32 experts' 5-dim outputs and
                # reduce over time within each batch
                s_sb = constp.tile([TGT, BPC], F32, tag="s")
                for n in range(NT):
                    ps3 = psB.tile([TGT, NSZ], F32, tag="ps3")
                    for kt in range(8):
                        nc.tensor.matmul(
                            ps3[:],
                            we3_sb[:, kt, :],
                            h2_sb[:, kt, n * NSZ:(n + 1) * NSZ],
                            start=(kt == 0), stop=(kt == 7))
                    nc.vector.reduce_sum(
                        s_sb[:, 2 * n:2 * n + 2],
                        ps3[:].rearrange("p (g t) -> p g t", t=T),
                        axis=mybir.AxisListType.X)
                nc.sync.dma_start(out_d[:], s_sb[:])

    nc.compile()
    return nc


def _prep(inputs):
    """Host-side: quantize/cast, transpose feats to [feature, token], build
    per-core input maps."""
    f32 = np.float32

    def bf(x):
        return np.asarray(x, f32).astype(NPBF)

    def q8(x, s=1.0):
        return (np.asarray(x, f32) * f32(s)).astype(NPF8)

    feats = {
        "v": np.asarray(inputs["video_feat"], f32).reshape(B * T, 768),
        "t": np.asarray(inputs["text_feat"], f32).reshape(B * T, 768),
        "a": np.asarray(inputs["audio_feat"], f32).reshape(B * T, 1024),
    }
    featsT = {k: q8(v.T) for k, v in feats.items()}

    wkeys = {"v": "Wv", "t": "Wt", "a": "Wa"}
    bkeys = {"v": "bv", "t": "bt", "a": "ba"}
    shared = {}
    for mn, kin, _, _ in MODS:
        shared[f"W{mn}"] = q8(inputs[wkeys[mn]], SW)
        shared[f"b{mn}"] = np.ascontiguousarray(
            np.asarray(inputs[bkeys[mn]], f32).reshape(GKT, 128).T)
    shared["Wp"] = q8(inputs["Wp"], SW)
    shared["bp"] = np.ascontiguousarray(
        np.asarray(inputs["bp"], f32).reshape(PKT, 128).T * f32(SC))
    shared["We1"] = q8(inputs["We1"], SW)
    shared["be1"] = np.ascontiguousarray(
        np.asarray(inputs["be1"], f32).reshape(E, PKT, 128).transpose(2, 0, 1))
    shared["We2"] = np.ascontiguousarray(
        bf(inputs["We2"]).reshape(E, PKT, 128, HID).transpose(2, 1, 0, 3))
    shared["be2"] = np.ascontiguousarray(np.asarray(inputs["be2"], f32).T)
    shared["We3"] = np.ascontiguousarray(
        bf(inputs["We3"]).reshape(8, 128, TGT).transpose(1, 0, 2))

    in_maps = []
    for c in range(N_CORES):
        m = dict(shared)
        sl = slice(c * TOK, (c + 1) * TOK)
        for mn, _, _, _ in MODS:
            m[f"x{mn}"] = np.ascontiguousarray(featsT[mn][:, sl])
        in_maps.append(m)
    be3_sum = np.asarray(inputs["be3"], f32).sum(axis=0)
    return in_maps, be3_sum


def kernel(**inputs):
    global _NC, LAST_RESULT
    if _NC is None:
        _NC = _build()
    in_maps, be3_sum = _prep(inputs)
    trace = bool(os.environ.get("BASS_KERNEL_TRACE"))
    kwargs = {}
    if trace:
        import concourse.bass_utils as _bu
        _bu.upload_artifacts = lambda d: d  # no artifact bucket here
        kwargs["tmpdir"] = os.environ.get("BASS_KERNEL_TRACE_DIR") or None
    res = run_bass_kernel_spmd(_NC, in_maps, list(range(N_CORES)),
                               trace=trace, **kwargs)
    LAST_RESULT = res
    logits = np.empty((B, TGT), np.float32)
    for c in range(N_CORES):
        s = res.results[c]["out"]  # [TGT, BPC]
        logits[c * BPC:(c + 1) * BPC] = ((s + be3_sum[:, None] * T) / (E * T)).T
    return logits


# revision 10
# speedup vs baseline: 1.9005x; 1.0126x over previous
"""Trainium2 Bass kernel for the GPT2Shared multimodal ensemble MLP.

Pipeline (per token): three modality adapters (Linear+GELU) -> shared
projection -> concat -> 32-expert ensemble MLP (2304->768->32->5, relu) ->
mean over experts -> mean over time.

Sharding: pure data-parallel over the batch dim. Each of the 8 cores gets
4 batches (1024 tokens) and runs the whole pipeline for its tokens; the
final reduction over experts+time happens on-device, so each core emits a
[5, 4] partial and the host only rescales/concats.

All on-device tensors live in [feature, token] layout so every matmul uses
the natural weight layout as the stationary operand and no transposes are
needed anywhere.

Precision: the adapter, projection and ensemble-L1 matmuls (97% of the
MACs) run in fp8 e4m3 with MatmulPerfMode.DoubleRow (2x PE throughput,
256-deep contraction per instruction). Weights are pre-scaled x32 on the
host so they sit in e4m3's normal range; the dequant is folded into each
activation's scale operand. The chunk activations are stored x4 in fp8
(folded the same way; relu/identity are homogeneous). The small ensemble
L2/L3 layers stay bf16 - they are <10% of the compute but dominate the
fp8 quantization error of the final logits.
"""

import os
import sys

for _p in ("/opt/trn_rl_repo", "/root/.axon_site/_ro/trn_rl_repo"):
    if os.path.isdir(_p) and _p not in sys.path:
        sys.path.append(_p)

import ml_dtypes
import numpy as np

import concourse.bass as bass
import concourse.tile as tile
from concourse import bacc, mybir
from concourse.bass_utils import run_bass_kernel_spmd

BF16 = mybir.dt.bfloat16
F8 = mybir.dt.float8e4
F32 = mybir.dt.float32
NPBF = ml_dtypes.bfloat16
NPF8 = ml_dtypes.float8_e4m3

N_CORES = 8
B, T = 32, 256
TOK = B * T // N_CORES          # 1024 tokens per core
BPC = B // N_CORES              # 4 batches per core
NT, NSZ = 2, 512                # token tiles per core
GKT = 18                        # 2304 gelu/chunk features = 18 k-tiles
PKT = 6                         # 768 proj features = 6 k-tiles
E, HID, TGT = 32, 32, 5
SW = 32.0                       # host-side fp8 weight pre-scale
SC = 4.0                        # fp8 storage scale of the chunk activations
# (name, in_dim, in_ktiles, chunk row-tile offset) in reference concat order
# chunk = [video, text, audio]
MODS = (("v", 768, 6, 0), ("t", 768, 6, 6), ("a", 1024, 8, 12))

_NC = None
LAST_RESULT = None


def _build():
    nc = bacc.Bacc("TRN2", target_bir_lowering=False, debug=False,
                   num_devices=N_CORES)

    dr = {}
    for mn, kin, _, _ in MODS:
        dr[f"x{mn}"] = nc.dram_tensor(f"x{mn}", [kin, TOK], F8, kind="ExternalInput")
        dr[f"W{mn}"] = nc.dram_tensor(f"W{mn}", [kin, 2304], F8, kind="ExternalInput")
        dr[f"b{mn}"] = nc.dram_tensor(f"b{mn}", [128, GKT], F32, kind="ExternalInput")
    dr["Wp"] = nc.dram_tensor("Wp", [2304, 768], F8, kind="ExternalInput")
    dr["bp"] = nc.dram_tensor("bp", [128, PKT], F32, kind="ExternalInput")  # 4*bp
    dr["We1"] = nc.dram_tensor("We1", [E, 2304, 768], F8, kind="ExternalInput")
    dr["be1"] = nc.dram_tensor("be1", [128, E, PKT], F32, kind="ExternalInput")
    # host-rearranged: [p, kt, e, h] <- We2[e, kt*128+p, h]
    dr["We2"] = nc.dram_tensor("We2", [128, PKT, E, HID], BF16, kind="ExternalInput")
    dr["be2"] = nc.dram_tensor("be2", [HID, E], F32, kind="ExternalInput")
    # host-stacked: [p, kt, t] <- We3[(kt*128+p)//32, (kt*128+p)%32, t]
    dr["We3"] = nc.dram_tensor("We3", [128, 8, TGT], BF16, kind="ExternalInput")
    out_d = nc.dram_tensor("out", [TGT, BPC], F32, kind="ExternalOutput")

    gelu = mybir.ActivationFunctionType.Gelu_apprx_tanh
    relu = mybir.ActivationFunctionType.Relu
    ident = mybir.ActivationFunctionType.Identity
    DR = mybir.MatmulPerfMode.DoubleRow

    with tile.TileContext(nc) as tc:
        with (
            tc.tile_pool(name="const", bufs=1) as constp,
            tc.tile_pool(name="persist", bufs=1) as perp,
            tc.tile_pool(name="psA", bufs=4, space=bass.MemorySpace.PSUM) as psA,
            tc.tile_pool(name="we1p", bufs=3) as we1p,
        ):
            # small constants
            be1_sb = constp.tile([128, E, PKT], F32, tag="be1")
            nc.sync.dma_start(be1_sb[:], dr["be1"][:])
            we2_sb = constp.tile([128, PKT, E, HID], BF16, tag="we2")
            nc.sync.dma_start(we2_sb[:], dr["We2"][:])
            be2_sb = constp.tile([HID, E], F32, tag="be2")
            nc.sync.dma_start(be2_sb[:], dr["be2"][:])
            we3_sb = constp.tile([128, 8, TGT], BF16, tag="we3")
            nc.sync.dma_start(we3_sb[:], dr["We3"][:])
            bp_sb = constp.tile([128, PKT], F32, tag="bp")
            nc.sync.dma_start(bp_sb[:], dr["bp"][:])

            chunk_sb = perp.tile([128, GKT, TOK], F8, tag="chunk")

            # ---------------- adapters + shared projection ----------------
            with (
                tc.tile_pool(name="adw", bufs=1) as adw,
                tc.tile_pool(name="adf", bufs=2) as adf,
            ):
                wp_sb = adw.tile([128, GKT, 768], F8, tag="wp")
                nc.sync.dma_start(
                    wp_sb[:], dr["Wp"].rearrange("(kt p) m -> p kt m", p=128))
                for mn, kin, kint, coff in MODS:
                    bm_sb = constp.tile([128, GKT], F32, tag=f"b{mn}")
                    nc.sync.dma_start(bm_sb[:], dr[f"b{mn}"][:])
                    wm_sb = adw.tile([128, 8, 2304], F8, tag="wmod")
                    nc.sync.dma_start(
                        wm_sb[:, :kint, :],
                        dr[f"W{mn}"].rearrange("(kt p) m -> p kt m", p=128))
                    f_sb = adf.tile([128, 8, TOK], F8, tag="feat")
                    nc.sync.dma_start(
                        f_sb[:, :kint, :],
                        dr[f"x{mn}"].rearrange("(kt p) n -> p kt n", p=128))
                    g_sb = adw.tile([128, GKT, TOK], F8, tag="g")
                    # g = gelu(x @ Wm + bm), in [feature, token] layout.
                    # n is the innermost matmul loop so each stationary
                    # weight block is reused for both token tiles.
                    for gf in range(GKT):
                        pss = [psA.tile([128, NSZ], F32, tag="ps", name=f"ps{n}")
                               for n in range(NT)]
                        for j in range(kint // 2):
                            for n in range(NT):
                                nc.tensor.matmul(
                                    pss[n][:],
                                    wm_sb[:, 2 * j:2 * j + 2,
                                          gf * 128:(gf + 1) * 128],
                                    f_sb[:, 2 * j:2 * j + 2,
                                         n * NSZ:(n + 1) * NSZ],
                                    start=(j == 0), stop=(j == kint // 2 - 1),
                                    perf_mode=DR)
                        for n in range(NT):
                            nc.scalar.activation(
                                g_sb[:, gf, n * NSZ:(n + 1) * NSZ], pss[n][:],
                                gelu, bias=bm_sb[:, gf:gf + 1], scale=1.0 / SW)
                    # chunk rows [coff:coff+6] = SC * (g @ Wp + bp)
                    for pf in range(PKT):
                        pss = [psA.tile([128, NSZ], F32, tag="ps", name=f"ps{n}")
                               for n in range(NT)]
                        for j in range(GKT // 2):
                            for n in range(NT):
                                nc.tensor.matmul(
                                    pss[n][:],
                                    wp_sb[:, 2 * j:2 * j + 2,
                                          pf * 128:(pf + 1) * 128],
                                    g_sb[:, 2 * j:2 * j + 2,
                                         n * NSZ:(n + 1) * NSZ],
                                    start=(j == 0), stop=(j == GKT // 2 - 1),
                                    perf_mode=DR)
                        for n in range(NT):
                            nc.scalar.activation(
                                chunk_sb[:, coff + pf, n * NSZ:(n + 1) * NSZ],
                                pss[n][:], ident, bias=bp_sb[:, pf:pf + 1],
                                scale=SC / SW)

            # ---------------- ensemble ----------------
            h2_sb = perp.tile([128, 8, TOK], BF16, tag="h2")
            with (
                tc.tile_pool(name="h1p", bufs=3) as h1p,
                tc.tile_pool(name="psB", bufs=2, space=bass.MemorySpace.PSUM) as psB,
            ):
                for e in range(E):
                    w1_sb = we1p.tile([128, GKT, 768], F8, tag="w1")
                    nc.sync.dma_start(
                        w1_sb[:],
                        dr["We1"][e].rearrange("(kt p) m -> p kt m", p=128))
                    h1_sb = h1p.tile([128, PKT, TOK], BF16, tag="h1")
                    for pf in range(PKT):
                        pss = [psA.tile([128, NSZ], F32, tag="ps", name=f"ps{n}")
                               for n in range(NT)]
                        for j in range(GKT // 2):
                            for n in range(NT):
                                nc.tensor.matmul(
                                    pss[n][:],
                                    w1_sb[:, 2 * j:2 * j + 2,
                                          pf * 128:(pf + 1) * 128],
                                    chunk_sb[:, 2 * j:2 * j + 2,
                                             n * NSZ:(n + 1) * NSZ],
                                    start=(j == 0), stop=(j == GKT // 2 - 1),
                                    perf_mode=DR)
                        for n in range(NT):
                            nc.scalar.activation(
                                h1_sb[:, pf, n * NSZ:(n + 1) * NSZ], pss[n][:],
                                relu, bias=be1_sb[:, e, pf:pf + 1],
                                scale=1.0 / (SW * SC))
                    ps2s = [psB.tile([HID, NSZ], F32, tag="ps2", name=f"ps2_{n}")
                            for n in range(NT)]
                    for kt in range(PKT):
                        for n in range(NT):
                            nc.tensor.matmul(
                                ps2s[n][:],
                                we2_sb[:, kt, e, :],
                                h1_sb[:, kt, n * NSZ:(n + 1) * NSZ],
                                start=(kt == 0), stop=(kt == PKT - 1))
                    q = e % 4
                    for n in range(NT):
                        nc.scalar.activation(
                            h2_sb[q * 32:(q + 1) * 32, e // 4,
                                  n * NSZ:(n + 1) * NSZ],
                            ps2s[n][:], relu, bias=be2_sb[:, e:e + 1])

                # ensemble head: accumulate all 32 experts' 5-dim outputs and
                # reduce over time within each batch
                s_sb = constp.tile([TGT, BPC], F32, tag="s")
                for n in range(NT):
                    ps3 = psB.tile([TGT, NSZ], F32, tag="ps3")
                    for kt in range(8):
                        nc.tensor.matmul(
                            ps3[:],
                            we3_sb[:, kt, :],
                            h2_sb[:, kt, n * NSZ:(n + 1) * NSZ],
                            start=(kt == 0), stop=(kt == 7))
                    nc.vector.reduce_sum(
                        s_sb[:, 2 * n:2 * n + 2],
                        ps3[:].rearrange("p (g t) -> p g t", t=T),
                        axis=mybir.AxisListType.X)
                nc.sync.dma_start(out_d[:], s_sb[:])

    nc.compile()
    return nc


def _prep(inputs):
    """Host-side: quantize/cast, transpose feats to [feature, token], build
    per-core input maps."""
    f32 = np.float32

    def bf(x):
        return np.asarray(x, f32).astype(NPBF)

    def q8(x, s=1.0):
        return (np.asarray(x, f32) * f32(s)).astype(NPF8)

    feats = {
        "v": np.asarray(inputs["video_feat"], f32).reshape(B * T, 768),
        "t": np.asarray(inputs["text_feat"], f32).reshape(B * T, 768),
        "a": np.asarray(inputs["audio_feat"], f32).reshape(B * T, 1024),
    }
    featsT = {k: q8(v.T) for k, v in feats.items()}

    wkeys = {"v": "Wv", "t": "Wt", "a": "Wa"}
    bkeys = {"v": "bv", "t": "bt", "a": "ba"}
    shared = {}
    for mn, kin, _, _ in MODS:
        shared[f"W{mn}"] = q8(inputs[wkeys[mn]], SW)
        shared[f"b{mn}"] = np.ascontiguousarray(
            np.asarray(inputs[bkeys[mn]], f32).reshape(GKT, 128).T)
    shared["Wp"] = q8(inputs["Wp"], SW)
    shared["bp"] = np.ascontiguousarray(
        np.asarray(inputs["bp"], f32).reshape(PKT, 128).T * f32(SC))
    shared["We1"] = q8(inputs["We1"], SW)
    shared["be1"] = np.ascontiguousarray(
        np.asarray(inputs["be1"], f32).reshape(E, PKT, 128).transpose(2, 0, 1))
    shared["We2"] = np.ascontiguousarray(
        bf(inputs["We2"]).reshape(E, PKT, 128, HID).transpose(2, 1, 0, 3))
    shared["be2"] = np.ascontiguousarray(np.asarray(inputs["be2"], f32).T)
    shared["We3"] = np.ascontiguousarray(
        bf(inputs["We3"]).reshape(8, 128, TGT).transpose(1, 0, 2))

    in_maps = []
    for c in range(N_CORES):
        m = dict(shared)
        sl = slice(c * TOK, (c + 1) * TOK)
        for mn, _, _, _ in MODS:
            m[f"x{mn}"] = np.ascontiguousarray(featsT[mn][:, sl])
        in_maps.append(m)
    be3_sum = np.asarray(inputs["be3"], f32).sum(axis=0)
    return in_maps, be3_sum


def kernel(**inputs):
    global _NC, LAST_RESULT
    if _NC is None:
        _NC = _build()
    in_maps, be3_sum = _prep(inputs)
    trace = bool(os.environ.get("BASS_KERNEL_TRACE"))
    kwargs = {}
    if trace:
        import concourse.bass_utils as _bu
        _bu.upload_artifacts = lambda d: d  # no artifact bucket here
        kwargs["tmpdir"] = os.environ.get("BASS_KERNEL_TRACE_DIR") or None
    res = run_bass_kernel_spmd(_NC, in_maps, list(range(N_CORES)),
                               trace=trace, **kwargs)
    LAST_RESULT = res
    logits = np.empty((B, TGT), np.float32)
    for c in range(N_CORES):
        s = res.results[c]["out"]  # [TGT, BPC]
        logits[c * BPC:(c + 1) * BPC] = ((s + be3_sum[:, None] * T) / (E * T)).T
    return logits


# revision 11
# speedup vs baseline: 1.9101x; 1.0050x over previous
"""Trainium2 Bass kernel for the GPT2Shared multimodal ensemble MLP.

Pipeline (per token): three modality adapters (Linear+GELU) -> shared
projection -> concat -> 32-expert ensemble MLP (2304->768->32->5, relu) ->
mean over experts -> mean over time.

Sharding: pure data-parallel over the batch dim. Each of the 8 cores gets
4 batches (1024 tokens) and runs the whole pipeline for its tokens; the
final reduction over experts+time happens on-device, so each core emits a
[5, 4] partial and the host only rescales/concats.

All on-device tensors live in [feature, token] layout so every matmul uses
the natural weight layout as the stationary operand and no transposes are
needed anywhere.

Precision: the adapter, projection and ensemble-L1 matmuls (97% of the
MACs) run in fp8 e4m3 with MatmulPerfMode.DoubleRow (2x PE throughput,
256-deep contraction per instruction). Weights are pre-scaled x32 on the
host so they sit in e4m3's normal range; the dequant is folded into each
activation's scale operand. The chunk activations are stored x4 in fp8
(folded the same way; relu/identity are homogeneous). The small ensemble
L2/L3 layers stay bf16 - they are <10% of the compute but dominate the
fp8 quantization error of the final logits.
"""

import os
import sys

for _p in ("/opt/trn_rl_repo", "/root/.axon_site/_ro/trn_rl_repo"):
    if os.path.isdir(_p) and _p not in sys.path:
        sys.path.append(_p)

import ml_dtypes
import numpy as np

import concourse.bass as bass
import concourse.tile as tile
from concourse import bacc, mybir
from concourse.bass_utils import run_bass_kernel_spmd

BF16 = mybir.dt.bfloat16
F8 = mybir.dt.float8e4
F32 = mybir.dt.float32
NPBF = ml_dtypes.bfloat16
NPF8 = ml_dtypes.float8_e4m3

N_CORES = 8
B, T = 32, 256
TOK = B * T // N_CORES          # 1024 tokens per core
BPC = B // N_CORES              # 4 batches per core
NT, NSZ = 2, 512                # token tiles per core
GKT = 18                        # 2304 gelu/chunk features = 18 k-tiles
PKT = 6                         # 768 proj features = 6 k-tiles
E, HID, TGT = 32, 32, 5
SW = 32.0                       # host-side fp8 weight pre-scale
SC = 4.0                        # fp8 storage scale of the chunk activations
# (name, in_dim, in_ktiles, chunk row-tile offset) in reference concat order
# chunk = [video, text, audio]
MODS = (("v", 768, 6, 0), ("t", 768, 6, 6), ("a", 1024, 8, 12))

_NC = None
LAST_RESULT = None


def _build():
    nc = bacc.Bacc("TRN2", target_bir_lowering=False, debug=False,
                   num_devices=N_CORES)

    dr = {}
    for mn, kin, _, _ in MODS:
        dr[f"x{mn}"] = nc.dram_tensor(f"x{mn}", [kin, TOK], F8, kind="ExternalInput")
        dr[f"W{mn}"] = nc.dram_tensor(f"W{mn}", [kin, 2304], F8, kind="ExternalInput")
        dr[f"b{mn}"] = nc.dram_tensor(f"b{mn}", [128, GKT], F32, kind="ExternalInput")
    dr["Wp"] = nc.dram_tensor("Wp", [2304, 768], F8, kind="ExternalInput")
    dr["bp"] = nc.dram_tensor("bp", [128, PKT], F32, kind="ExternalInput")  # 4*bp
    dr["We1"] = nc.dram_tensor("We1", [E, 2304, 768], F8, kind="ExternalInput")
    dr["be1"] = nc.dram_tensor("be1", [128, E, PKT], F32, kind="ExternalInput")
    # host-rearranged: [p, kt, e, h] <- We2[e, kt*128+p, h]
    dr["We2"] = nc.dram_tensor("We2", [128, PKT, E, HID], BF16, kind="ExternalInput")
    dr["be2"] = nc.dram_tensor("be2", [HID, E], F32, kind="ExternalInput")
    # host-stacked: [p, kt, t] <- We3[(kt*128+p)//32, (kt*128+p)%32, t]
    dr["We3"] = nc.dram_tensor("We3", [128, 8, TGT], BF16, kind="ExternalInput")
    out_d = nc.dram_tensor("out", [TGT, BPC], F32, kind="ExternalOutput")

    gelu = mybir.ActivationFunctionType.Gelu_apprx_tanh
    relu = mybir.ActivationFunctionType.Relu
    ident = mybir.ActivationFunctionType.Identity
    DR = mybir.MatmulPerfMode.DoubleRow

    with tile.TileContext(nc) as tc:
        with (
            tc.tile_pool(name="const", bufs=1) as constp,
            tc.tile_pool(name="persist", bufs=1) as perp,
            tc.tile_pool(name="psA", bufs=4, space=bass.MemorySpace.PSUM) as psA,
            tc.tile_pool(name="we1p", bufs=3) as we1p,
        ):
            chunk_sb = perp.tile([128, GKT, TOK], F8, tag="chunk")

            # ---------------- adapters + shared projection ----------------
            with (
                tc.tile_pool(name="adw", bufs=1) as adw,
                tc.tile_pool(name="adwm", bufs=2) as adwm,
                tc.tile_pool(name="adf", bufs=2) as adf,
            ):
                bm_sbs, wm_sbs, f_sbs = {}, {}, {}

                def issue_mod_dma(mn, kint):
                    bm_sbs[mn] = constp.tile([128, GKT], F32, tag=f"b{mn}",
                                             name=f"bm_{mn}")
                    nc.sync.dma_start(bm_sbs[mn][:], dr[f"b{mn}"][:])
                    wm_sbs[mn] = adwm.tile([128, 8, 2304], F8, tag="wmod",
                                           name=f"wm_{mn}")
                    nc.sync.dma_start(
                        wm_sbs[mn][:, :kint, :],
                        dr[f"W{mn}"].rearrange("(kt p) m -> p kt m", p=128))
                    f_sbs[mn] = adf.tile([128, 8, TOK], F8, tag="feat",
                                         name=f"f_{mn}")
                    nc.sync.dma_start(
                        f_sbs[mn][:, :kint, :],
                        dr[f"x{mn}"].rearrange("(kt p) n -> p kt n", p=128))

                # first modality's inputs lead the DMA queue: the kernel's
                # first matmul depends on them, everything else can trail
                issue_mod_dma(MODS[0][0], MODS[0][2])

                be1_sb = constp.tile([128, E, PKT], F32, tag="be1")
                nc.sync.dma_start(be1_sb[:], dr["be1"][:])
                we2_sb = constp.tile([128, PKT, E, HID], BF16, tag="we2")
                nc.sync.dma_start(we2_sb[:], dr["We2"][:])
                be2_sb = constp.tile([HID, E], F32, tag="be2")
                nc.sync.dma_start(be2_sb[:], dr["be2"][:])
                we3_sb = constp.tile([128, 8, TGT], BF16, tag="we3")
                nc.sync.dma_start(we3_sb[:], dr["We3"][:])
                bp_sb = constp.tile([128, PKT], F32, tag="bp")
                nc.sync.dma_start(bp_sb[:], dr["bp"][:])

                wp_sb = adw.tile([128, GKT, 768], F8, tag="wp")
                nc.sync.dma_start(
                    wp_sb[:], dr["Wp"].rearrange("(kt p) m -> p kt m", p=128))
                for mn, kin, kint, coff in MODS:
                    if mn not in wm_sbs:
                        issue_mod_dma(mn, kint)
                    bm_sb, wm_sb, f_sb = bm_sbs[mn], wm_sbs[mn], f_sbs[mn]
                    g_sb = adw.tile([128, GKT, TOK], F8, tag="g")
                    # g = gelu(x @ Wm + bm), in [feature, token] layout.
                    # n is the innermost matmul loop so each stationary
                    # weight block is reused for both token tiles.
                    for gf in range(GKT):
                        pss = [psA.tile([128, NSZ], F32, tag="ps", name=f"ps{n}")
                               for n in range(NT)]
                        for j in range(kint // 2):
                            for n in range(NT):
                                nc.tensor.matmul(
                                    pss[n][:],
                                    wm_sb[:, 2 * j:2 * j + 2,
                                          gf * 128:(gf + 1) * 128],
                                    f_sb[:, 2 * j:2 * j + 2,
                                         n * NSZ:(n + 1) * NSZ],
                                    start=(j == 0), stop=(j == kint // 2 - 1),
                                    perf_mode=DR)
                        for n in range(NT):
                            nc.scalar.activation(
                                g_sb[:, gf, n * NSZ:(n + 1) * NSZ], pss[n][:],
                                gelu, bias=bm_sb[:, gf:gf + 1], scale=1.0 / SW)
                    # chunk rows [coff:coff+6] = SC * (g @ Wp + bp)
                    for pf in range(PKT):
                        pss = [psA.tile([128, NSZ], F32, tag="ps", name=f"ps{n}")
                               for n in range(NT)]
                        for j in range(GKT // 2):
                            for n in range(NT):
                                nc.tensor.matmul(
                                    pss[n][:],
                                    wp_sb[:, 2 * j:2 * j + 2,
                                          pf * 128:(pf + 1) * 128],
                                    g_sb[:, 2 * j:2 * j + 2,
                                         n * NSZ:(n + 1) * NSZ],
                                    start=(j == 0), stop=(j == GKT // 2 - 1),
                                    perf_mode=DR)
                        for n in range(NT):
                            nc.scalar.activation(
                                chunk_sb[:, coff + pf, n * NSZ:(n + 1) * NSZ],
                                pss[n][:], ident, bias=bp_sb[:, pf:pf + 1],
                                scale=SC / SW)

            # ---------------- ensemble ----------------
            h2_sb = perp.tile([128, 8, TOK], BF16, tag="h2")
            with (
                tc.tile_pool(name="h1p", bufs=3) as h1p,
                tc.tile_pool(name="psB", bufs=2, space=bass.MemorySpace.PSUM) as psB,
            ):
                for e in range(E):
                    w1_sb = we1p.tile([128, GKT, 768], F8, tag="w1")
                    nc.sync.dma_start(
                        w1_sb[:],
                        dr["We1"][e].rearrange("(kt p) m -> p kt m", p=128))
                    h1_sb = h1p.tile([128, PKT, TOK], BF16, tag="h1")
                    for pf in range(PKT):
                        pss = [psA.tile([128, NSZ], F32, tag="ps", name=f"ps{n}")
                               for n in range(NT)]
                        for j in range(GKT // 2):
                            for n in range(NT):
                                nc.tensor.matmul(
                                    pss[n][:],
                                    w1_sb[:, 2 * j:2 * j + 2,
                                          pf * 128:(pf + 1) * 128],
                                    chunk_sb[:, 2 * j:2 * j + 2,
                                             n * NSZ:(n + 1) * NSZ],
                                    start=(j == 0), stop=(j == GKT // 2 - 1),
                                    perf_mode=DR)
                        for n in range(NT):
                            nc.scalar.activation(
                                h1_sb[:, pf, n * NSZ:(n + 1) * NSZ], pss[n][:],
                                relu, bias=be1_sb[:, e, pf:pf + 1],
                                scale=1.0 / (SW * SC))
                    ps2s = [psB.tile([HID, NSZ], F32, tag="ps2", name=f"ps2_{n}")
                            for n in range(NT)]
                    for kt in range(PKT):
                        for n in range(NT):
                            nc.tensor.matmul(
                                ps2s[n][:],
                                we2_sb[:, kt, e, :],
                                h1_sb[:, kt, n * NSZ:(n + 1) * NSZ],
                                start=(kt == 0), stop=(kt == PKT - 1))
                    q = e % 4
                    for n in range(NT):
                        nc.scalar.activation(
                            h2_sb[q * 32:(q + 1) * 32, e // 4,
                                  n * NSZ:(n + 1) * NSZ],
                            ps2s[n][:], relu, bias=be2_sb[:, e:e + 1])

                # ensemble head: accumulate all 32 experts' 5-dim outputs and
                # reduce over time within each batch
                s_sb = constp.tile([TGT, BPC], F32, tag="s")
                for n in range(NT):
                    ps3 = psB.tile([TGT, NSZ], F32, tag="ps3")
                    for kt in range(8):
                        nc.tensor.matmul(
                            ps3[:],
                            we3_sb[:, kt, :],
                            h2_sb[:, kt, n * NSZ:(n + 1) * NSZ],
                            start=(kt == 0), stop=(kt == 7))
                    nc.vector.reduce_sum(
                        s_sb[:, 2 * n:2 * n + 2],
                        ps3[:].rearrange("p (g t) -> p g t", t=T),
                        axis=mybir.AxisListType.X)
                nc.sync.dma_start(out_d[:], s_sb[:])

    nc.compile()
    return nc


def _prep(inputs):
    """Host-side: quantize/cast, transpose feats to [feature, token], build
    per-core input maps."""
    f32 = np.float32

    def bf(x):
        return np.asarray(x, f32).astype(NPBF)

    def q8(x, s=1.0):
        return (np.asarray(x, f32) * f32(s)).astype(NPF8)

    feats = {
        "v": np.asarray(inputs["video_feat"], f32).reshape(B * T, 768),
        "t": np.asarray(inputs["text_feat"], f32).reshape(B * T, 768),
        "a": np.asarray(inputs["audio_feat"], f32).reshape(B * T, 1024),
    }
    featsT = {k: q8(v.T) for k, v in feats.items()}

    wkeys = {"v": "Wv", "t": "Wt", "a": "Wa"}
    bkeys = {"v": "bv", "t": "bt", "a": "ba"}
    shared = {}
    for mn, kin, _, _ in MODS:
        shared[f"W{mn}"] = q8(inputs[wkeys[mn]], SW)
        shared[f"b{mn}"] = np.ascontiguousarray(
            np.asarray(inputs[bkeys[mn]], f32).reshape(GKT, 128).T)
    shared["Wp"] = q8(inputs["Wp"], SW)
    shared["bp"] = np.ascontiguousarray(
        np.asarray(inputs["bp"], f32).reshape(PKT, 128).T * f32(SC))
    shared["We1"] = q8(inputs["We1"], SW)
    shared["be1"] = np.ascontiguousarray(
        np.asarray(inputs["be1"], f32).reshape(E, PKT, 128).transpose(2, 0, 1))
    shared["We2"] = np.ascontiguousarray(
        bf(inputs["We2"]).reshape(E, PKT, 128, HID).transpose(2, 1, 0, 3))
    shared["be2"] = np.ascontiguousarray(np.asarray(inputs["be2"], f32).T)
    shared["We3"] = np.ascontiguousarray(
        bf(inputs["We3"]).reshape(8, 128, TGT).transpose(1, 0, 2))

    in_maps = []
    for c in range(N_CORES):
        m = dict(shared)
        sl = slice(c * TOK, (c + 1) * TOK)
        for mn, _, _, _ in MODS:
            m[f"x{mn}"] = np.ascontiguousarray(featsT[mn][:, sl])
        in_maps.append(m)
    be3_sum = np.asarray(inputs["be3"], f32).sum(axis=0)
    return in_maps, be3_sum


def kernel(**inputs):
    global _NC, LAST_RESULT
    if _NC is None:
        _NC = _build()
    in_maps, be3_sum = _prep(inputs)
    trace = bool(os.environ.get("BASS_KERNEL_TRACE"))
    kwargs = {}
    if trace:
        import concourse.bass_utils as _bu
        _bu.upload_artifacts = lambda d: d  # no artifact bucket here
        kwargs["tmpdir"] = os.environ.get("BASS_KERNEL_TRACE_DIR") or None
    res = run_bass_kernel_spmd(_NC, in_maps, list(range(N_CORES)),
                               trace=trace, **kwargs)
    LAST_RESULT = res
    logits = np.empty((B, TGT), np.float32)
    for c in range(N_CORES):
        s = res.results[c]["out"]  # [TGT, BPC]
        logits[c * BPC:(c + 1) * BPC] = ((s + be3_sum[:, None] * T) / (E * T)).T
    return logits
